# revision 64
# baseline (speedup 1.0000x reference)
# Trainium2 Bass kernel for nn_CompanionMatrixBlock.
#
# Model: AR(16) companion-matrix block. x: (128, 512, 64) fp32.
#   y_horizon (128, 128, 64): 128-step linear recurrence s <- Ch @ s on the
#     1024-dim companion state, emitting the first 64 coords each step.
#   y_back (128, 512, 64): sliding-window einsum == 16-tap block convolution
#     of the time-reversed series with W_b.
#
# Device strategy (8 NeuronCores):
#   * y_horizon: y_h[b,t] = G_t @ s0[b] with G_t = W_h @ Ch^t depending only
#     on weights. The 128 G matrices are host-precomputed (weight prep);
#     the horizon dim is sharded across cores: core c computes
#     (128 x 1024) @ (1024 x 1024) for steps 16c..16c+15 at full PE util.
#   * y_back: batch-sharded (16 samples/core). All 16 lags are covered by
#     4 matmuls/sample at full 128x128 utilization: the K dim packs two
#     time-shifts (dual-shifted data copy, +4 steps), the M dim packs two
#     lag-groups whose outputs land 8 columns apart; the scalar engine
#     stages group A to SBUF and one DVE add folds group B. (Outputs
#     t>=504 need no fold: the lag-8..15 terms there hit the zero pad.)
#   Operands are bf16 (fp32 PSUM accumulation): measured rel.err ~2.4e-3.
#   All DRAM tensors are packed 128-partition-major so every load/store is
#   a large contiguous DMA.

import numpy as np
import ml_dtypes

import concourse.bacc as bacc
import concourse.mybir as mybir
from concourse.tile import TileContext
from concourse.bass_utils import run_bass_kernel_spmd

B = 128          # batch
BACK = 512       # back horizon
N = 64           # series dim
LAGS = 16
NL = N * LAGS    # 1024 companion state dim
HOR = 128        # horizon
NCORES = 8
BLOC = B // NCORES      # 16 samples/core for y_back
HLOC = HOR // NCORES    # 16 horizon steps/core
XTW = 516        # XT width: stream slices reach col 3+512 = 515

DT = mybir.dt.bfloat16
NPDT = ml_dtypes.bfloat16

_CACHE = {}


def _build_nc():
    nc = bacc.Bacc()
    # xt[r, 524*s + tau]: dual-shifted x^T, 4 samples per DMA group
    xt = nc.declare_dram_parameter("xt", [128, BLOC * XTW], DT, isOutput=False)
    # s0t packed (128, 8*128): chunk k at columns 128k
    s0t = nc.declare_dram_parameter("s0t", [128, 8 * 128], DT, isOutput=False)
    # gslt packed (128, 8*1024): K-chunk k at columns 1024k
    gslt = nc.declare_dram_parameter("gslt", [128, 8 * NL], DT, isOutput=False)
    # wbq packed (128, 4*128)
    wbq = nc.declare_dram_parameter("wbq", [128, 4 * 128], DT, isOutput=False)
    # ybt: 4 quads, each (128, 1024): quad q pair w in cols 512w, sample
    # 4q+2w+(r//64), row r%64 = n, col t
    # outputs are stored bf16 (adds ~5e-4 rel err, halves store traffic)
    ybt = nc.declare_dram_parameter("ybt", [4, 128, 1024], DT, isOutput=True)
    yh = nc.declare_dram_parameter("yh", [B, NL], DT, isOutput=True)

    f32 = mybir.dt.float32
    with TileContext(nc) as tc:
        with (
            tc.tile_pool(name="consts", bufs=1) as cpool,
            tc.tile_pool(name="xtp", bufs=1) as xpool,
            tc.tile_pool(name="gslp", bufs=4) as gpool,
            tc.tile_pool(name="ybp", bufs=4) as ybpool,
            tc.tile_pool(name="yhp", bufs=1) as yhpool,
            tc.tile_pool(name="ps1p", bufs=4, space="PSUM") as ps1pool,
            tc.tile_pool(name="pshp", bufs=1, space="PSUM") as pshpool,
        ):
            # PE warm-up: dummy matmuls on zeroed SBUF while the first input
            # DMAs are in flight, so real matmuls start at the warm clock
            # (HAM releases the throttle after ~3.4us of sustained activity).
            wu = cpool.tile([128, 1], DT, tag="wu")
            nc.gpsimd.memset(wu[:], 0)
            wu_ps = pshpool.tile([128, 512], f32, tag="ph0")
            for _ in range(7):
                nc.tensor.matmul(
                    wu_ps[:],
                    lhsT=wu[:].broadcast_to([128, 128]),
                    rhs=wu[:].broadcast_to([128, 512]),
                    start=True, stop=True,
                )

            # loads in emission order = DMA priority: weights first (small),
            # then x in 1/1/2/4-sample groups so the first matmul starts
            # early (remaining groups stream in under compute below).
            wbq_sb = cpool.tile([128, 4 * 128], DT, tag="wbq")
            nc.sync.dma_start(wbq_sb[:], wbq[:])
            # all 16 samples in ONE tile (16.8KB/partition), loaded by group
            # DMAs; Tile tracks sub-region deps, so sample-s matmuls only
            # wait on their own group's DMA.
            xgroups = [(0, 1), (1, 1), (2, 2), (4, 4), (8, 4), (12, 4)]
            xs = xpool.tile([128, BLOC * XTW], DT, tag="xtall")
            for gi in range(4):
                lo, n = xgroups[gi]
                nc.sync.dma_start(
                    xs[:, XTW * lo : XTW * (lo + n)], xt[:, XTW * lo : XTW * (lo + n)]
                )
            g2 = gpool.tile([128, 2 * NL], DT, tag="gsl")
            nc.sync.dma_start(g2[:], gslt[:, 0 : 2 * NL])
            gsl_sb = [g2]
            s0_sb = None

            # ---- y_back: 16 samples; output quads of 4 samples ----
            yb4s = []
            for quad in range(4):
                yb4 = ybpool.tile([128, 1024], DT, tag="yb4")
                yb4s.append(yb4)
                for w in range(2):
                    for half in range(2):
                        s = 4 * quad + 2 * w + half
                        c0 = XTW * s
                        ps1 = ps1pool.tile([128, 512], f32, tag="ps1")
                        for p in range(4):
                            nc.tensor.matmul(
                                ps1[:],
                                lhsT=wbq_sb[:, 128 * p : 128 * (p + 1)],
                                rhs=xs[:, c0 + p : c0 + p + 512],
                                start=(p == 0),
                                stop=(p == 3),
                            )
                        # y^T[n,t] = ps1[n,t] + ps1[64+n,t+8]  (t <= 503)
                        # for t >= 504 the lag-8..15 terms hit the zero pad,
                        # so the staged copy alone is already correct there
                        r0, t0 = 64 * half, 512 * w
                        nc.scalar.copy(
                            yb4[r0 : r0 + 64, t0 : t0 + 512], ps1[0:64, 0:512]
                        )
                        nc.vector.tensor_add(
                            yb4[r0 : r0 + 64, t0 : t0 + 504],
                            yb4[r0 : r0 + 64, t0 : t0 + 504],
                            ps1[64:128, 8:512],
                        )
                # stream in later inputs under this quad's compute
                if quad < 2:
                    lo, n = xgroups[quad + 4]
                    nc.sync.dma_start(
                        xs[:, XTW * lo : XTW * (lo + n)],
                        xt[:, XTW * lo : XTW * (lo + n)],
                    )
                if quad < 3:
                    g2 = gpool.tile([128, 2 * NL], DT, tag="gsl")
                    nc.sync.dma_start(
                        g2[:], gslt[:, 2 * NL * (quad + 1) : 2 * NL * (quad + 2)]
                    )
                    gsl_sb.append(g2)
                if quad == 0:
                    s0_sb = cpool.tile([128, 8 * 128], DT, tag="s0")
                    nc.sync.dma_start(s0_sb[:], s0t[:])
                nc.sync.dma_start(ybt[quad], yb4[:])

            # ---- y_horizon: yh = s0 @ gslt  (K=1024 accumulation) ----
            ph0 = pshpool.tile([128, 512], f32, tag="ph0")
            ph1 = pshpool.tile([128, 512], f32, tag="ph1")
            ph1b = pshpool.tile([128, 256], f32, tag="ph1b")
            yh_sb = yhpool.tile([128, NL], DT, tag="yh")
            for q, (yc, width, ph) in enumerate(
                [(0, 512, ph0), (512, 256, ph1), (768, 256, ph1b)]
            ):
                for k in range(8):
                    rhs = gsl_sb[k // 2]
                    c0 = NL * (k % 2) + yc
                    nc.tensor.matmul(
                        ph[:, 0:width],
                        lhsT=s0_sb[:, 128 * k : 128 * (k + 1)],
                        rhs=rhs[:, c0 : c0 + width],
                        start=(k == 0),
                        stop=(k == 7),
                    )
                nc.vector.tensor_copy(
                    yh_sb[:, yc : yc + width], ph[:, 0:width]
                )
                nc.sync.dma_start(
                    yh[:, yc : yc + width], yh_sb[:, yc : yc + width]
                )
    nc.finalize()
    return nc


def _host_prep(x, W_h, W_b):
    """Build per-core device inputs. Weight-only prep (G stack, wbq) is
    independent of x."""
    x = np.ascontiguousarray(x, dtype=np.float32)
    W_h = np.asarray(W_h, dtype=np.float32)
    W_b = np.asarray(W_b, dtype=np.float32)

    # G_t = W_h @ Ch^t computed via the shift structure of the companion.
    Wh64 = W_h.astype(np.float64)
    G = Wh64.copy()
    gstack = np.empty((HOR, N, NL), np.float64)
    for t in range(HOR):
        gstack[t] = G
        G = G[:, :N] @ Wh64 + np.concatenate(
            [G[:, N:], np.zeros((N, N))], axis=1
        )

    # per-core horizon weight slice, packed (128, 8*1024):
    # gslt[r, 1024k + 64*tau + n] = G_{16c+tau}[n, 128k + r]
    gslts = []
    for c in range(NCORES):
        sl = gstack[HLOC * c : HLOC * (c + 1)]          # (16, 64, 1024)
        g = sl.transpose(2, 0, 1).reshape(NL, NL)       # (k, tau*64+n)
        g = g.reshape(8, 128, NL).transpose(1, 0, 2).reshape(128, 8 * NL)
        gslts.append(np.ascontiguousarray(g).astype(NPDT))

    # wbq packed (128, 4*128): wbq[64h+n, 128p + 64g + n_out]
    #   = W_b[n_out, 64*(p+4h+8g)+n]
    wbq = np.empty((4, 128, 128), np.float32)
    for p in range(4):
        for h in range(2):
            for g in range(2):
                j = p + 4 * h + 8 * g
                wbq[p, 64 * h : 64 * h + 64, 64 * g : 64 * g + 64] = W_b[
                    :, 64 * j : 64 * j + 64
                ].T
    wbq = wbq.transpose(1, 0, 2).reshape(128, 4 * 128).astype(NPDT)
    wbq = np.ascontiguousarray(wbq)

    xr = x[:, ::-1, :]                                   # (B, 512, 64)
    # s0t packed (128, 8*128): s0t[r, 128k + b] = s0[b, 128k + r]
    s0 = xr[:, :LAGS, :].reshape(B, NL)
    s0t = s0.T.reshape(8, 128, B).transpose(1, 0, 2).reshape(128, 8 * B)
    s0t = np.ascontiguousarray(s0t).astype(NPDT)

    # XT: rows 0-63 = xr^T, rows 64-127 = xr^T shifted by +4 steps
    xrp = np.zeros((B, 4 + XTW, N), np.float32)
    xrp[:, :BACK] = xr
    xts = np.concatenate(
        [xrp[:, 0:XTW, :].transpose(0, 2, 1), xrp[:, 4 : 4 + XTW, :].transpose(0, 2, 1)],
        axis=1,
    )                                                    # (B, 128, XTW)

    in_maps = []
    for c in range(NCORES):
        xc = xts[BLOC * c : BLOC * (c + 1)]              # (16, 128, XTW)
        xc = xc.transpose(1, 0, 2).reshape(128, BLOC * XTW)
        in_maps.append(
            {
                "xt": np.ascontiguousarray(xc).astype(NPDT),
                "s0t": s0t,
                "gslt": gslts[c],
                "wbq": wbq,
            }
        )
    return in_maps


def kernel(x, W_h, W_b):
    if "nc" not in _CACHE:
        _CACHE["nc"] = _build_nc()
    nc = _CACHE["nc"]

    in_maps = _host_prep(x, W_h, W_b)
    res = run_bass_kernel_spmd(nc, in_maps, list(range(NCORES)))

    y_horizon = np.empty((B, HOR, N), np.float32)
    y_back = np.empty((B, BACK, N), np.float32)
    for c in range(NCORES):
        out = res.results[c]
        y_horizon[:, HLOC * c : HLOC * (c + 1), :] = out["yh"].reshape(B, HLOC, N)
        # ybt (4, 128, 1024) -> [q, (half, n), (w, t)]
        yb = out["ybt"].reshape(4, 2, 64, 2, 512)        # q, half, n, w, t
        yb = yb.transpose(0, 3, 1, 4, 2)                 # q, w, half, t, n
        y_back[BLOC * c : BLOC * (c + 1)] = yb.reshape(BLOC, BACK, N)
    return y_horizon, y_back


# revision 65
# speedup vs baseline: 1.0055x; 1.0055x over previous
# Trainium2 Bass kernel for nn_CompanionMatrixBlock.
#
# Model: AR(16) companion-matrix block. x: (128, 512, 64) fp32.
#   y_horizon (128, 128, 64): 128-step linear recurrence s <- Ch @ s on the
#     1024-dim companion state, emitting the first 64 coords each step.
#   y_back (128, 512, 64): sliding-window einsum == 16-tap block convolution
#     of the time-reversed series with W_b.
#
# Device strategy (8 NeuronCores):
#   * y_horizon: y_h[b,t] = G_t @ s0[b] with G_t = W_h @ Ch^t depending only
#     on weights. The 128 G matrices are host-precomputed (weight prep);
#     the horizon dim is sharded across cores: core c computes
#     (128 x 1024) @ (1024 x 1024) for steps 16c..16c+15 at full PE util.
#   * y_back: batch-sharded (16 samples/core). All 16 lags are covered by
#     4 matmuls/sample at full 128x128 utilization: the K dim packs two
#     time-shifts (dual-shifted data copy, +4 steps), the M dim packs two
#     lag-groups whose outputs land 8 columns apart; the scalar engine
#     stages group A to SBUF and one DVE add folds group B. (Outputs
#     t>=504 need no fold: the lag-8..15 terms there hit the zero pad.)
#   Operands are bf16 (fp32 PSUM accumulation): measured rel.err ~2.4e-3.
#   All DRAM tensors are packed 128-partition-major so every load/store is
#   a large contiguous DMA.

import numpy as np
import ml_dtypes

import concourse.bacc as bacc
import concourse.mybir as mybir
from concourse.tile import TileContext
from concourse.bass_utils import run_bass_kernel_spmd

B = 128          # batch
BACK = 512       # back horizon
N = 64           # series dim
LAGS = 16
NL = N * LAGS    # 1024 companion state dim
HOR = 128        # horizon
NCORES = 8
BLOC = B // NCORES      # 16 samples/core for y_back
HLOC = HOR // NCORES    # 16 horizon steps/core
XTW = 516        # XT width: stream slices reach col 3+512 = 515

DT = mybir.dt.bfloat16
NPDT = ml_dtypes.bfloat16

_CACHE = {}


def _build_nc():
    nc = bacc.Bacc()
    # xt[r, :512] = wbq weights; xt[r, 512 + 516*s + tau] = dual-shifted x^T
    # (weights ride in the first DMA so one descriptor unblocks the first MM)
    xt = nc.declare_dram_parameter("xt", [128, 512 + BLOC * XTW], DT, isOutput=False)
    # s0t packed (128, 8*128): chunk k at columns 128k
    s0t = nc.declare_dram_parameter("s0t", [128, 8 * 128], DT, isOutput=False)
    # gslt packed (128, 8*1024): K-chunk k at columns 1024k
    gslt = nc.declare_dram_parameter("gslt", [128, 8 * NL], DT, isOutput=False)
    # ybt: 4 quads, each (128, 1024): quad q pair w in cols 512w, sample
    # 4q+2w+(r//64), row r%64 = n, col t
    # outputs are stored bf16 (adds ~5e-4 rel err, halves store traffic)
    ybt = nc.declare_dram_parameter("ybt", [4, 128, 1024], DT, isOutput=True)
    yh = nc.declare_dram_parameter("yh", [B, NL], DT, isOutput=True)

    f32 = mybir.dt.float32
    with TileContext(nc) as tc:
        with (
            tc.tile_pool(name="consts", bufs=1) as cpool,
            tc.tile_pool(name="xtp", bufs=1) as xpool,
            tc.tile_pool(name="gslp", bufs=4) as gpool,
            tc.tile_pool(name="ybp", bufs=4) as ybpool,
            tc.tile_pool(name="yhp", bufs=1) as yhpool,
            tc.tile_pool(name="ps1p", bufs=4, space="PSUM") as ps1pool,
            tc.tile_pool(name="pshp", bufs=1, space="PSUM") as pshpool,
        ):
            # PE warm-up: dummy matmuls on zeroed SBUF while the first input
            # DMAs are in flight, so real matmuls start at the warm clock
            # (HAM releases the throttle after ~3.4us of sustained activity).
            wu = cpool.tile([128, 1], DT, tag="wu")
            nc.gpsimd.memset(wu[:], 0)
            wu_ps = pshpool.tile([128, 512], f32, tag="ph0")
            for _ in range(7):
                nc.tensor.matmul(
                    wu_ps[:],
                    lhsT=wu[:].broadcast_to([128, 128]),
                    rhs=wu[:].broadcast_to([128, 512]),
                    start=True, stop=True,
                )

            # weights + all 16 samples in ONE tile (17.5KB/partition),
            # loaded by group DMAs (first group carries wbq + sample 0);
            # Tile tracks sub-region deps, so sample-s matmuls only wait on
            # their own group's DMA.
            xgroups = [(0, 1), (1, 1), (2, 2), (4, 4), (8, 4), (12, 4)]
            xs = xpool.tile([128, 512 + BLOC * XTW], DT, tag="xtall")
            wbq_sb = xs
            nc.sync.dma_start(xs[:, 0 : 512 + XTW], xt[:, 0 : 512 + XTW])
            for gi in range(1, 4):
                lo, n = xgroups[gi]
                nc.sync.dma_start(
                    xs[:, 512 + XTW * lo : 512 + XTW * (lo + n)],
                    xt[:, 512 + XTW * lo : 512 + XTW * (lo + n)],
                )
            g2 = gpool.tile([128, 2 * NL], DT, tag="gsl")
            nc.sync.dma_start(g2[:], gslt[:, 0 : 2 * NL])
            gsl_sb = [g2]
            s0_sb = None

            # ---- y_back: 16 samples; output quads of 4 samples ----
            yb4s = []
            for quad in range(4):
                yb4 = ybpool.tile([128, 1024], DT, tag="yb4")
                yb4s.append(yb4)
                for w in range(2):
                    for half in range(2):
                        s = 4 * quad + 2 * w + half
                        c0 = 512 + XTW * s
                        ps1 = ps1pool.tile([128, 512], f32, tag="ps1")
                        for p in range(4):
                            nc.tensor.matmul(
                                ps1[:],
                                lhsT=wbq_sb[:, 128 * p : 128 * (p + 1)],
                                rhs=xs[:, c0 + p : c0 + p + 512],
                                start=(p == 0),
                                stop=(p == 3),
                            )
                        # y^T[n,t] = ps1[n,t] + ps1[64+n,t+8]  (t <= 503)
                        # for t >= 504 the lag-8..15 terms hit the zero pad,
                        # so the staged copy alone is already correct there
                        r0, t0 = 64 * half, 512 * w
                        nc.scalar.copy(
                            yb4[r0 : r0 + 64, t0 : t0 + 512], ps1[0:64, 0:512]
                        )
                        nc.vector.tensor_add(
                            yb4[r0 : r0 + 64, t0 : t0 + 504],
                            yb4[r0 : r0 + 64, t0 : t0 + 504],
                            ps1[64:128, 8:512],
                        )
                # stream in later inputs under this quad's compute
                if quad < 2:
                    lo, n = xgroups[quad + 4]
                    nc.sync.dma_start(
                        xs[:, 512 + XTW * lo : 512 + XTW * (lo + n)],
                        xt[:, 512 + XTW * lo : 512 + XTW * (lo + n)],
                    )
                if quad < 3:
                    g2 = gpool.tile([128, 2 * NL], DT, tag="gsl")
                    nc.sync.dma_start(
                        g2[:], gslt[:, 2 * NL * (quad + 1) : 2 * NL * (quad + 2)]
                    )
                    gsl_sb.append(g2)
                if quad == 0:
                    s0_sb = cpool.tile([128, 8 * 128], DT, tag="s0")
                    nc.sync.dma_start(s0_sb[:], s0t[:])
                nc.sync.dma_start(ybt[quad], yb4[:])

            # ---- y_horizon: yh = s0 @ gslt  (K=1024 accumulation) ----
            ph0 = pshpool.tile([128, 512], f32, tag="ph0")
            ph1 = pshpool.tile([128, 512], f32, tag="ph1")
            ph1b = pshpool.tile([128, 256], f32, tag="ph1b")
            yh_sb = yhpool.tile([128, NL], DT, tag="yh")
            for q, (yc, width, ph) in enumerate(
                [(0, 512, ph0), (512, 256, ph1), (768, 256, ph1b)]
            ):
                for k in range(8):
                    rhs = gsl_sb[k // 2]
                    c0 = NL * (k % 2) + yc
                    nc.tensor.matmul(
                        ph[:, 0:width],
                        lhsT=s0_sb[:, 128 * k : 128 * (k + 1)],
                        rhs=rhs[:, c0 : c0 + width],
                        start=(k == 0),
                        stop=(k == 7),
                    )
                nc.vector.tensor_copy(
                    yh_sb[:, yc : yc + width], ph[:, 0:width]
                )
                nc.sync.dma_start(
                    yh[:, yc : yc + width], yh_sb[:, yc : yc + width]
                )
    nc.finalize()
    return nc


def _host_prep(x, W_h, W_b):
    """Build per-core device inputs. Weight-only prep (G stack, wbq) is
    independent of x."""
    x = np.ascontiguousarray(x, dtype=np.float32)
    W_h = np.asarray(W_h, dtype=np.float32)
    W_b = np.asarray(W_b, dtype=np.float32)

    # G_t = W_h @ Ch^t computed via the shift structure of the companion.
    Wh64 = W_h.astype(np.float64)
    G = Wh64.copy()
    gstack = np.empty((HOR, N, NL), np.float64)
    for t in range(HOR):
        gstack[t] = G
        G = G[:, :N] @ Wh64 + np.concatenate(
            [G[:, N:], np.zeros((N, N))], axis=1
        )

    # per-core horizon weight slice, packed (128, 8*1024):
    # gslt[r, 1024k + 64*tau + n] = G_{16c+tau}[n, 128k + r]
    gslts = []
    for c in range(NCORES):
        sl = gstack[HLOC * c : HLOC * (c + 1)]          # (16, 64, 1024)
        g = sl.transpose(2, 0, 1).reshape(NL, NL)       # (k, tau*64+n)
        g = g.reshape(8, 128, NL).transpose(1, 0, 2).reshape(128, 8 * NL)
        gslts.append(np.ascontiguousarray(g).astype(NPDT))

    # wbq packed (128, 4*128): wbq[64h+n, 128p + 64g + n_out]
    #   = W_b[n_out, 64*(p+4h+8g)+n]
    wbq = np.empty((4, 128, 128), np.float32)
    for p in range(4):
        for h in range(2):
            for g in range(2):
                j = p + 4 * h + 8 * g
                wbq[p, 64 * h : 64 * h + 64, 64 * g : 64 * g + 64] = W_b[
                    :, 64 * j : 64 * j + 64
                ].T
    wbq = wbq.transpose(1, 0, 2).reshape(128, 4 * 128).astype(NPDT)
    wbq = np.ascontiguousarray(wbq)

    xr = x[:, ::-1, :]                                   # (B, 512, 64)
    # s0t packed (128, 8*128): s0t[r, 128k + b] = s0[b, 128k + r]
    s0 = xr[:, :LAGS, :].reshape(B, NL)
    s0t = s0.T.reshape(8, 128, B).transpose(1, 0, 2).reshape(128, 8 * B)
    s0t = np.ascontiguousarray(s0t).astype(NPDT)

    # XT: rows 0-63 = xr^T, rows 64-127 = xr^T shifted by +4 steps
    xrp = np.zeros((B, 4 + XTW, N), np.float32)
    xrp[:, :BACK] = xr
    xts = np.concatenate(
        [xrp[:, 0:XTW, :].transpose(0, 2, 1), xrp[:, 4 : 4 + XTW, :].transpose(0, 2, 1)],
        axis=1,
    )                                                    # (B, 128, XTW)

    in_maps = []
    for c in range(NCORES):
        xc = xts[BLOC * c : BLOC * (c + 1)]              # (16, 128, XTW)
        xc = xc.transpose(1, 0, 2).reshape(128, BLOC * XTW)
        xtw = np.concatenate([wbq.astype(np.float32), xc], axis=1)
        in_maps.append(
            {
                "xt": np.ascontiguousarray(xtw).astype(NPDT),
                "s0t": s0t,
                "gslt": gslts[c],
            }
        )
    return in_maps


def kernel(x, W_h, W_b):
    if "nc" not in _CACHE:
        _CACHE["nc"] = _build_nc()
    nc = _CACHE["nc"]

    in_maps = _host_prep(x, W_h, W_b)
    res = run_bass_kernel_spmd(nc, in_maps, list(range(NCORES)))

    y_horizon = np.empty((B, HOR, N), np.float32)
    y_back = np.empty((B, BACK, N), np.float32)
    for c in range(NCORES):
        out = res.results[c]
        y_horizon[:, HLOC * c : HLOC * (c + 1), :] = out["yh"].reshape(B, HLOC, N)
        # ybt (4, 128, 1024) -> [q, (half, n), (w, t)]
        yb = out["ybt"].reshape(4, 2, 64, 2, 512)        # q, half, n, w, t
        yb = yb.transpose(0, 3, 1, 4, 2)                 # q, w, half, t, n
        y_back[BLOC * c : BLOC * (c + 1)] = yb.reshape(BLOC, BACK, N)
    return y_horizon, y_back


# revision 69
# speedup vs baseline: 1.0135x; 1.0080x over previous
# Trainium2 Bass kernel for nn_CompanionMatrixBlock.
#
# Model: AR(16) companion-matrix block. x: (128, 512, 64) fp32.
#   y_horizon (128, 128, 64): 128-step linear recurrence s <- Ch @ s on the
#     1024-dim companion state, emitting the first 64 coords each step.
#   y_back (128, 512, 64): sliding-window einsum == 16-tap block convolution
#     of the time-reversed series with W_b.
#
# Device strategy (8 NeuronCores):
#   * y_horizon: y_h[b,t] = G_t @ s0[b] with G_t = W_h @ Ch^t depending only
#     on weights. The 128 G matrices are host-precomputed (weight prep);
#     the horizon dim is sharded across cores: core c computes
#     (128 x 1024) @ (1024 x 1024) for steps 16c..16c+15 at full PE util.
#   * y_back: batch-sharded (16 samples/core). All 16 lags are covered by
#     4 matmuls/sample at full 128x128 utilization: the K dim packs two
#     time-shifts (dual-shifted data copy, +4 steps), the M dim packs two
#     lag-groups whose outputs land 8 columns apart; the scalar engine
#     stages group A to SBUF and one DVE add folds group B. (Outputs
#     t>=504 need no fold: the lag-8..15 terms there hit the zero pad.)
#   Operands are bf16 (fp32 PSUM accumulation): measured rel.err ~2.4e-3.
#   All DRAM tensors are packed 128-partition-major so every load/store is
#   a large contiguous DMA.

import numpy as np
import ml_dtypes

import concourse.bacc as bacc
import concourse.mybir as mybir
from concourse.tile import TileContext
from concourse.bass_utils import run_bass_kernel_spmd

B = 128          # batch
BACK = 512       # back horizon
N = 64           # series dim
LAGS = 16
NL = N * LAGS    # 1024 companion state dim
HOR = 128        # horizon
NCORES = 8
BLOC = B // NCORES      # 16 samples/core for y_back
HLOC = HOR // NCORES    # 16 horizon steps/core
XTW = 516        # XT width: stream slices reach col 3+512 = 515

DT = mybir.dt.bfloat16
NPDT = ml_dtypes.bfloat16

_CACHE = {}


def _build_nc():
    nc = bacc.Bacc()
    # xt[r, :512] = wbq weights; xt[r, 512 + 516*s + tau] = dual-shifted x^T
    # (weights ride in the first DMA so one descriptor unblocks the first MM)
    xt = nc.declare_dram_parameter("xt", [128, 512 + BLOC * XTW], DT, isOutput=False)
    # s0t packed (128, 8*128): chunk k at columns 128k
    s0t = nc.declare_dram_parameter("s0t", [128, 8 * 128], DT, isOutput=False)
    # gslt packed (128, 8*1024): K-chunk k at columns 1024k
    gslt = nc.declare_dram_parameter("gslt", [128, 8 * NL], DT, isOutput=False)
    # ybt: 4 quads, each (128, 1024): quad q pair w in cols 512w, sample
    # 4q+2w+(r//64), row r%64 = n, col t
    # outputs are stored bf16 (adds ~5e-4 rel err, halves store traffic)
    ybt = nc.declare_dram_parameter("ybt", [4, 128, 1024], DT, isOutput=True)
    yh = nc.declare_dram_parameter("yh", [B, NL], DT, isOutput=True)

    f32 = mybir.dt.float32
    with TileContext(nc) as tc:
        with (
            tc.tile_pool(name="consts", bufs=1) as cpool,
            tc.tile_pool(name="xtp", bufs=1) as xpool,
            tc.tile_pool(name="gslp", bufs=4) as gpool,
            tc.tile_pool(name="ybp", bufs=4) as ybpool,
            tc.tile_pool(name="yhp", bufs=1) as yhpool,
            tc.tile_pool(name="ps1p", bufs=4, space="PSUM") as ps1pool,
            tc.tile_pool(name="pshp", bufs=1, space="PSUM") as pshpool,
        ):
            # PE warm-up: dummy matmuls on zeroed SBUF while the first input
            # DMAs are in flight, so real matmuls start at the warm clock
            # (HAM releases the throttle after ~3.4us of sustained activity).
            wu = cpool.tile([128, 1], DT, tag="wu")
            nc.gpsimd.memset(wu[:], 0)
            wu_ps = pshpool.tile([128, 512], f32, tag="ph0")
            for _ in range(6):
                nc.tensor.matmul(
                    wu_ps[:],
                    lhsT=wu[:].broadcast_to([128, 128]),
                    rhs=wu[:].broadcast_to([128, 512]),
                    start=True, stop=True,
                )

            # weights + all 16 samples in ONE tile (17.5KB/partition),
            # loaded by group DMAs (first group carries wbq + sample 0);
            # Tile tracks sub-region deps, so sample-s matmuls only wait on
            # their own group's DMA.
            xgroups = [(0, 1), (1, 1), (2, 2), (4, 4), (8, 4), (12, 4)]
            xs = xpool.tile([128, 512 + BLOC * XTW], DT, tag="xtall")
            wbq_sb = xs
            nc.sync.dma_start(xs[:, 0 : 512 + XTW], xt[:, 0 : 512 + XTW])
            for gi in range(1, 4):
                lo, n = xgroups[gi]
                nc.sync.dma_start(
                    xs[:, 512 + XTW * lo : 512 + XTW * (lo + n)],
                    xt[:, 512 + XTW * lo : 512 + XTW * (lo + n)],
                )
            g2 = gpool.tile([128, 2 * NL], DT, tag="gsl")
            nc.sync.dma_start(g2[:], gslt[:, 0 : 2 * NL])
            gsl_sb = [g2]
            s0_sb = None

            # ---- y_back: 16 samples; output quads of 4 samples ----
            yb4s = []
            for quad in range(4):
                yb4 = ybpool.tile([128, 1024], DT, tag="yb4")
                yb4s.append(yb4)
                for w in range(2):
                    for half in range(2):
                        s = 4 * quad + 2 * w + half
                        c0 = 512 + XTW * s
                        ps1 = ps1pool.tile([128, 512], f32, tag="ps1")
                        for p in range(4):
                            nc.tensor.matmul(
                                ps1[:],
                                lhsT=wbq_sb[:, 128 * p : 128 * (p + 1)],
                                rhs=xs[:, c0 + p : c0 + p + 512],
                                start=(p == 0),
                                stop=(p == 3),
                            )
                        # y^T[n,t] = ps1[n,t] + ps1[64+n,t+8]  (t <= 503)
                        # for t >= 504 the lag-8..15 terms hit the zero pad,
                        # so the staged copy alone is already correct there
                        r0, t0 = 64 * half, 512 * w
                        nc.scalar.copy(
                            yb4[r0 : r0 + 64, t0 : t0 + 512], ps1[0:64, 0:512]
                        )
                        nc.vector.tensor_add(
                            yb4[r0 : r0 + 64, t0 : t0 + 504],
                            yb4[r0 : r0 + 64, t0 : t0 + 504],
                            ps1[64:128, 8:512],
                        )
                # stream in later inputs under this quad's compute
                if quad < 2:
                    lo, n = xgroups[quad + 4]
                    nc.sync.dma_start(
                        xs[:, 512 + XTW * lo : 512 + XTW * (lo + n)],
                        xt[:, 512 + XTW * lo : 512 + XTW * (lo + n)],
                    )
                if quad < 3:
                    g2 = gpool.tile([128, 2 * NL], DT, tag="gsl")
                    nc.sync.dma_start(
                        g2[:], gslt[:, 2 * NL * (quad + 1) : 2 * NL * (quad + 2)]
                    )
                    gsl_sb.append(g2)
                if quad == 0:
                    s0_sb = cpool.tile([128, 8 * 128], DT, tag="s0")
                    nc.sync.dma_start(s0_sb[:], s0t[:])
                nc.sync.dma_start(ybt[quad], yb4[:])

            # ---- y_horizon: yh = s0 @ gslt  (K=1024 accumulation) ----
            ph0 = pshpool.tile([128, 512], f32, tag="ph0")
            ph1 = pshpool.tile([128, 512], f32, tag="ph1")
            ph1b = pshpool.tile([128, 256], f32, tag="ph1b")
            yh_sb = yhpool.tile([128, NL], DT, tag="yh")
            for q, (yc, width, ph) in enumerate(
                [(0, 512, ph0), (512, 256, ph1), (768, 256, ph1b)]
            ):
                for k in range(8):
                    rhs = gsl_sb[k // 2]
                    c0 = NL * (k % 2) + yc
                    nc.tensor.matmul(
                        ph[:, 0:width],
                        lhsT=s0_sb[:, 128 * k : 128 * (k + 1)],
                        rhs=rhs[:, c0 : c0 + width],
                        start=(k == 0),
                        stop=(k == 7),
                    )
                nc.vector.tensor_copy(
                    yh_sb[:, yc : yc + width], ph[:, 0:width]
                )
                nc.sync.dma_start(
                    yh[:, yc : yc + width], yh_sb[:, yc : yc + width]
                )
    nc.finalize()
    return nc


def _host_prep(x, W_h, W_b):
    """Build per-core device inputs. Weight-only prep (G stack, wbq) is
    independent of x."""
    x = np.ascontiguousarray(x, dtype=np.float32)
    W_h = np.asarray(W_h, dtype=np.float32)
    W_b = np.asarray(W_b, dtype=np.float32)

    # G_t = W_h @ Ch^t computed via the shift structure of the companion.
    Wh64 = W_h.astype(np.float64)
    G = Wh64.copy()
    gstack = np.empty((HOR, N, NL), np.float64)
    for t in range(HOR):
        gstack[t] = G
        G = G[:, :N] @ Wh64 + np.concatenate(
            [G[:, N:], np.zeros((N, N))], axis=1
        )

    # per-core horizon weight slice, packed (128, 8*1024):
    # gslt[r, 1024k + 64*tau + n] = G_{16c+tau}[n, 128k + r]
    gslts = []
    for c in range(NCORES):
        sl = gstack[HLOC * c : HLOC * (c + 1)]          # (16, 64, 1024)
        g = sl.transpose(2, 0, 1).reshape(NL, NL)       # (k, tau*64+n)
        g = g.reshape(8, 128, NL).transpose(1, 0, 2).reshape(128, 8 * NL)
        gslts.append(np.ascontiguousarray(g).astype(NPDT))

    # wbq packed (128, 4*128): wbq[64h+n, 128p + 64g + n_out]
    #   = W_b[n_out, 64*(p+4h+8g)+n]
    wbq = np.empty((4, 128, 128), np.float32)
    for p in range(4):
        for h in range(2):
            for g in range(2):
                j = p + 4 * h + 8 * g
                wbq[p, 64 * h : 64 * h + 64, 64 * g : 64 * g + 64] = W_b[
                    :, 64 * j : 64 * j + 64
                ].T
    wbq = wbq.transpose(1, 0, 2).reshape(128, 4 * 128).astype(NPDT)
    wbq = np.ascontiguousarray(wbq)

    xr = x[:, ::-1, :]                                   # (B, 512, 64)
    # s0t packed (128, 8*128): s0t[r, 128k + b] = s0[b, 128k + r]
    s0 = xr[:, :LAGS, :].reshape(B, NL)
    s0t = s0.T.reshape(8, 128, B).transpose(1, 0, 2).reshape(128, 8 * B)
    s0t = np.ascontiguousarray(s0t).astype(NPDT)

    # XT: rows 0-63 = xr^T, rows 64-127 = xr^T shifted by +4 steps
    xrp = np.zeros((B, 4 + XTW, N), np.float32)
    xrp[:, :BACK] = xr
    xts = np.concatenate(
        [xrp[:, 0:XTW, :].transpose(0, 2, 1), xrp[:, 4 : 4 + XTW, :].transpose(0, 2, 1)],
        axis=1,
    )                                                    # (B, 128, XTW)

    in_maps = []
    for c in range(NCORES):
        xc = xts[BLOC * c : BLOC * (c + 1)]              # (16, 128, XTW)
        xc = xc.transpose(1, 0, 2).reshape(128, BLOC * XTW)
        xtw = np.concatenate([wbq.astype(np.float32), xc], axis=1)
        in_maps.append(
            {
                "xt": np.ascontiguousarray(xtw).astype(NPDT),
                "s0t": s0t,
                "gslt": gslts[c],
            }
        )
    return in_maps


def kernel(x, W_h, W_b):
    if "nc" not in _CACHE:
        _CACHE["nc"] = _build_nc()
    nc = _CACHE["nc"]

    in_maps = _host_prep(x, W_h, W_b)
    res = run_bass_kernel_spmd(nc, in_maps, list(range(NCORES)))

    y_horizon = np.empty((B, HOR, N), np.float32)
    y_back = np.empty((B, BACK, N), np.float32)
    for c in range(NCORES):
        out = res.results[c]
        y_horizon[:, HLOC * c : HLOC * (c + 1), :] = out["yh"].reshape(B, HLOC, N)
        # ybt (4, 128, 1024) -> [q, (half, n), (w, t)]
        yb = out["ybt"].reshape(4, 2, 64, 2, 512)        # q, half, n, w, t
        yb = yb.transpose(0, 3, 1, 4, 2)                 # q, w, half, t, n
        y_back[BLOC * c : BLOC * (c + 1)] = yb.reshape(BLOC, BACK, N)
    return y_horizon, y_back


# revision 71
# speedup vs baseline: 1.0156x; 1.0020x over previous
# Trainium2 Bass kernel for nn_CompanionMatrixBlock.
#
# Model: AR(16) companion-matrix block. x: (128, 512, 64) fp32.
#   y_horizon (128, 128, 64): 128-step linear recurrence s <- Ch @ s on the
#     1024-dim companion state, emitting the first 64 coords each step.
#   y_back (128, 512, 64): sliding-window einsum == 16-tap block convolution
#     of the time-reversed series with W_b.
#
# Device strategy (8 NeuronCores):
#   * y_horizon: y_h[b,t] = G_t @ s0[b] with G_t = W_h @ Ch^t depending only
#     on weights. The 128 G matrices are host-precomputed (weight prep);
#     the horizon dim is sharded across cores: core c computes
#     (128 x 1024) @ (1024 x 1024) for steps 16c..16c+15 at full PE util.
#   * y_back: batch-sharded (16 samples/core). All 16 lags are covered by
#     4 matmuls/sample at full 128x128 utilization: the K dim packs two
#     time-shifts (dual-shifted data copy, +4 steps), the M dim packs two
#     lag-groups whose outputs land 8 columns apart; the scalar engine
#     stages group A to SBUF and one DVE add folds group B. (Outputs
#     t>=504 need no fold: the lag-8..15 terms there hit the zero pad.)
#   Operands are bf16 (fp32 PSUM accumulation): measured rel.err ~2.4e-3.
#   All DRAM tensors are packed 128-partition-major so every load/store is
#   a large contiguous DMA.

import numpy as np
import ml_dtypes

import concourse.bacc as bacc
import concourse.mybir as mybir
from concourse.tile import TileContext
from concourse.bass_utils import run_bass_kernel_spmd

B = 128          # batch
BACK = 512       # back horizon
N = 64           # series dim
LAGS = 16
NL = N * LAGS    # 1024 companion state dim
HOR = 128        # horizon
NCORES = 8
BLOC = B // NCORES      # 16 samples/core for y_back
HLOC = HOR // NCORES    # 16 horizon steps/core
XTW = 516        # XT width: stream slices reach col 3+512 = 515

DT = mybir.dt.bfloat16
NPDT = ml_dtypes.bfloat16

_CACHE = {}


def _build_nc():
    nc = bacc.Bacc()
    # xt[r, :512] = wbq weights; xt[r, 512 + 516*s + tau] = dual-shifted x^T
    # (weights ride in the first DMA so one descriptor unblocks the first MM)
    xt = nc.declare_dram_parameter("xt", [128, 512 + BLOC * XTW], DT, isOutput=False)
    # s0t packed (128, 8*128): chunk k at columns 128k
    s0t = nc.declare_dram_parameter("s0t", [128, 8 * 128], DT, isOutput=False)
    # gslt packed (128, 8*1024): K-chunk k at columns 1024k
    gslt = nc.declare_dram_parameter("gslt", [128, 8 * NL], DT, isOutput=False)
    # ybt: 4 quads, each (128, 1024): quad q pair w in cols 512w, sample
    # 4q+2w+(r//64), row r%64 = n, col t
    # outputs are stored bf16 (adds ~5e-4 rel err, halves store traffic)
    ybt = nc.declare_dram_parameter("ybt", [4, 128, 1024], DT, isOutput=True)
    yh = nc.declare_dram_parameter("yh", [B, NL], DT, isOutput=True)

    f32 = mybir.dt.float32
    with TileContext(nc) as tc:
        with (
            tc.tile_pool(name="consts", bufs=1) as cpool,
            tc.tile_pool(name="xtp", bufs=1) as xpool,
            tc.tile_pool(name="gslp", bufs=4) as gpool,
            tc.tile_pool(name="ybp", bufs=4) as ybpool,
            tc.tile_pool(name="yhp", bufs=1) as yhpool,
            tc.tile_pool(name="ps1p", bufs=4, space="PSUM") as ps1pool,
            tc.tile_pool(name="pshp", bufs=1, space="PSUM") as pshpool,
        ):
            # PE warm-up: dummy matmuls on zeroed SBUF while the first input
            # DMAs are in flight, so real matmuls start at the warm clock
            # (HAM releases the throttle after ~3.4us of sustained activity).
            wu = cpool.tile([128, 1], DT, tag="wu")
            nc.vector.memset(wu[:], 0)
            wu_ps = pshpool.tile([128, 512], f32, tag="ph0")
            for _ in range(6):
                nc.tensor.matmul(
                    wu_ps[:],
                    lhsT=wu[:].broadcast_to([128, 128]),
                    rhs=wu[:].broadcast_to([128, 512]),
                    start=True, stop=True,
                )

            # weights + all 16 samples in ONE tile (17.5KB/partition),
            # loaded by group DMAs (first group carries wbq + sample 0);
            # Tile tracks sub-region deps, so sample-s matmuls only wait on
            # their own group's DMA.
            xgroups = [(0, 1), (1, 1), (2, 2), (4, 4), (8, 4), (12, 4)]
            xs = xpool.tile([128, 512 + BLOC * XTW], DT, tag="xtall")
            wbq_sb = xs
            nc.sync.dma_start(xs[:, 0 : 512 + XTW], xt[:, 0 : 512 + XTW])
            for gi in range(1, 4):
                lo, n = xgroups[gi]
                nc.sync.dma_start(
                    xs[:, 512 + XTW * lo : 512 + XTW * (lo + n)],
                    xt[:, 512 + XTW * lo : 512 + XTW * (lo + n)],
                )
            g2 = gpool.tile([128, 2 * NL], DT, tag="gsl")
            nc.sync.dma_start(g2[:], gslt[:, 0 : 2 * NL])
            gsl_sb = [g2]
            s0_sb = None

            # ---- y_back: 16 samples; output quads of 4 samples ----
            yb4s = []
            for quad in range(4):
                yb4 = ybpool.tile([128, 1024], DT, tag="yb4")
                yb4s.append(yb4)
                for w in range(2):
                    for half in range(2):
                        s = 4 * quad + 2 * w + half
                        c0 = 512 + XTW * s
                        ps1 = ps1pool.tile([128, 512], f32, tag="ps1")
                        for p in range(4):
                            nc.tensor.matmul(
                                ps1[:],
                                lhsT=wbq_sb[:, 128 * p : 128 * (p + 1)],
                                rhs=xs[:, c0 + p : c0 + p + 512],
                                start=(p == 0),
                                stop=(p == 3),
                            )
                        # y^T[n,t] = ps1[n,t] + ps1[64+n,t+8]  (t <= 503)
                        # for t >= 504 the lag-8..15 terms hit the zero pad,
                        # so the staged copy alone is already correct there
                        r0, t0 = 64 * half, 512 * w
                        nc.scalar.copy(
                            yb4[r0 : r0 + 64, t0 : t0 + 512], ps1[0:64, 0:512]
                        )
                        nc.vector.tensor_add(
                            yb4[r0 : r0 + 64, t0 : t0 + 504],
                            yb4[r0 : r0 + 64, t0 : t0 + 504],
                            ps1[64:128, 8:512],
                        )
                # stream in later inputs under this quad's compute
                if quad < 2:
                    lo, n = xgroups[quad + 4]
                    nc.sync.dma_start(
                        xs[:, 512 + XTW * lo : 512 + XTW * (lo + n)],
                        xt[:, 512 + XTW * lo : 512 + XTW * (lo + n)],
                    )
                if quad < 3:
                    g2 = gpool.tile([128, 2 * NL], DT, tag="gsl")
                    nc.sync.dma_start(
                        g2[:], gslt[:, 2 * NL * (quad + 1) : 2 * NL * (quad + 2)]
                    )
                    gsl_sb.append(g2)
                if quad == 0:
                    s0_sb = cpool.tile([128, 8 * 128], DT, tag="s0")
                    nc.sync.dma_start(s0_sb[:], s0t[:])
                nc.sync.dma_start(ybt[quad], yb4[:])

            # ---- y_horizon: yh = s0 @ gslt  (K=1024 accumulation) ----
            ph0 = pshpool.tile([128, 512], f32, tag="ph0")
            ph1 = pshpool.tile([128, 512], f32, tag="ph1")
            ph1b = pshpool.tile([128, 256], f32, tag="ph1b")
            yh_sb = yhpool.tile([128, NL], DT, tag="yh")
            for q, (yc, width, ph) in enumerate(
                [(0, 512, ph0), (512, 256, ph1), (768, 256, ph1b)]
            ):
                for k in range(8):
                    rhs = gsl_sb[k // 2]
                    c0 = NL * (k % 2) + yc
                    nc.tensor.matmul(
                        ph[:, 0:width],
                        lhsT=s0_sb[:, 128 * k : 128 * (k + 1)],
                        rhs=rhs[:, c0 : c0 + width],
                        start=(k == 0),
                        stop=(k == 7),
                    )
                nc.vector.tensor_copy(
                    yh_sb[:, yc : yc + width], ph[:, 0:width]
                )
                nc.sync.dma_start(
                    yh[:, yc : yc + width], yh_sb[:, yc : yc + width]
                )
    nc.finalize()
    return nc


def _host_prep(x, W_h, W_b):
    """Build per-core device inputs. Weight-only prep (G stack, wbq) is
    independent of x."""
    x = np.ascontiguousarray(x, dtype=np.float32)
    W_h = np.asarray(W_h, dtype=np.float32)
    W_b = np.asarray(W_b, dtype=np.float32)

    # G_t = W_h @ Ch^t computed via the shift structure of the companion.
    Wh64 = W_h.astype(np.float64)
    G = Wh64.copy()
    gstack = np.empty((HOR, N, NL), np.float64)
    for t in range(HOR):
        gstack[t] = G
        G = G[:, :N] @ Wh64 + np.concatenate(
            [G[:, N:], np.zeros((N, N))], axis=1
        )

    # per-core horizon weight slice, packed (128, 8*1024):
    # gslt[r, 1024k + 64*tau + n] = G_{16c+tau}[n, 128k + r]
    gslts = []
    for c in range(NCORES):
        sl = gstack[HLOC * c : HLOC * (c + 1)]          # (16, 64, 1024)
        g = sl.transpose(2, 0, 1).reshape(NL, NL)       # (k, tau*64+n)
        g = g.reshape(8, 128, NL).transpose(1, 0, 2).reshape(128, 8 * NL)
        gslts.append(np.ascontiguousarray(g).astype(NPDT))

    # wbq packed (128, 4*128): wbq[64h+n, 128p + 64g + n_out]
    #   = W_b[n_out, 64*(p+4h+8g)+n]
    wbq = np.empty((4, 128, 128), np.float32)
    for p in range(4):
        for h in range(2):
            for g in range(2):
                j = p + 4 * h + 8 * g
                wbq[p, 64 * h : 64 * h + 64, 64 * g : 64 * g + 64] = W_b[
                    :, 64 * j : 64 * j + 64
                ].T
    wbq = wbq.transpose(1, 0, 2).reshape(128, 4 * 128).astype(NPDT)
    wbq = np.ascontiguousarray(wbq)

    xr = x[:, ::-1, :]                                   # (B, 512, 64)
    # s0t packed (128, 8*128): s0t[r, 128k + b] = s0[b, 128k + r]
    s0 = xr[:, :LAGS, :].reshape(B, NL)
    s0t = s0.T.reshape(8, 128, B).transpose(1, 0, 2).reshape(128, 8 * B)
    s0t = np.ascontiguousarray(s0t).astype(NPDT)

    # XT: rows 0-63 = xr^T, rows 64-127 = xr^T shifted by +4 steps
    xrp = np.zeros((B, 4 + XTW, N), np.float32)
    xrp[:, :BACK] = xr
    xts = np.concatenate(
        [xrp[:, 0:XTW, :].transpose(0, 2, 1), xrp[:, 4 : 4 + XTW, :].transpose(0, 2, 1)],
        axis=1,
    )                                                    # (B, 128, XTW)

    in_maps = []
    for c in range(NCORES):
        xc = xts[BLOC * c : BLOC * (c + 1)]              # (16, 128, XTW)
        xc = xc.transpose(1, 0, 2).reshape(128, BLOC * XTW)
        xtw = np.concatenate([wbq.astype(np.float32), xc], axis=1)
        in_maps.append(
            {
                "xt": np.ascontiguousarray(xtw).astype(NPDT),
                "s0t": s0t,
                "gslt": gslts[c],
            }
        )
    return in_maps


def kernel(x, W_h, W_b):
    if "nc" not in _CACHE:
        _CACHE["nc"] = _build_nc()
    nc = _CACHE["nc"]

    in_maps = _host_prep(x, W_h, W_b)
    res = run_bass_kernel_spmd(nc, in_maps, list(range(NCORES)))

    y_horizon = np.empty((B, HOR, N), np.float32)
    y_back = np.empty((B, BACK, N), np.float32)
    for c in range(NCORES):
        out = res.results[c]
        y_horizon[:, HLOC * c : HLOC * (c + 1), :] = out["yh"].reshape(B, HLOC, N)
        # ybt (4, 128, 1024) -> [q, (half, n), (w, t)]
        yb = out["ybt"].reshape(4, 2, 64, 2, 512)        # q, half, n, w, t
        yb = yb.transpose(0, 3, 1, 4, 2)                 # q, w, half, t, n
        y_back[BLOC * c : BLOC * (c + 1)] = yb.reshape(BLOC, BACK, N)
    return y_horizon, y_back
